# revision 9
# baseline (speedup 1.0000x reference)
"""Trainium2 Bass kernel for nn_AggregateVideo (segment_reduce).

Algorithm (per video, fully on-device):
  csum1[i] = inclusive cumsum of mask            (1-based valid count)
  n        = csum1[L-1]
  s_k      = min(rne_round(k*n/T), n-1)          k = 0..T   (exact fp32)
  S_t=s_t, E_t=s_{t+1}, H_t=max(E_t, S_t+1)
  W[i,t]   = (S_t < c_i) * (c_i <= H_t)          c_i = csum1[i]*mask[i]  (0/1 matrix)
  out[t,:] = (W.T @ feats)[t,:] / (H_t - S_t)

The W bucket ranges exactly reproduce the reference's compaction + bucket-mean
(incl. degenerate s==e buckets picking the single row s: H=S+1 there).

Sharding: batch dim B=64 across 8 cores, 8 videos per core, zero communication.
"""

from contextlib import ExitStack

import numpy as np

B, L, D = 64, 4096, 256
T = 256
N_CORES = 8
VPC = B // N_CORES  # videos per core
P = 128
NCH = L // P  # 32 chunks per video
TWO23 = float(2**23)
SIG = 10000.0  # sigmoid step sharpness (saturates exactly to 0/1)

_CACHE = {}


def _build_program(gp_stride=3, gp_misc=True):
    import concourse.mybir as mybir
    import concourse.tile as tile
    from concourse import bacc

    f32 = mybir.dt.float32
    f32r = mybir.dt.float32r
    u8 = mybir.dt.uint8
    Alu = mybir.AluOpType
    Act = mybir.ActivationFunctionType

    nc = bacc.Bacc(
        "TRN2",
        target_bir_lowering=False,
        debug=False,
        num_devices=N_CORES,
    )

    feats_d = nc.dram_tensor("feats", [VPC, L, D], f32r, kind="ExternalInput").ap()
    masks_d = nc.dram_tensor("masks", [VPC, L], u8, kind="ExternalInput").ap()
    tconst_d = nc.dram_tensor("tconst", [P, T + 1], f32, kind="ExternalInput").ap()
    ident_d = nc.dram_tensor("ident", [P, P], f32, kind="ExternalInput").ap()
    triex_d = nc.dram_tensor("triex", [32, 32], f32, kind="ExternalInput").ap()
    ones_d = nc.dram_tensor("ones32", [32, P], f32, kind="ExternalInput").ap()
    out_d = nc.dram_tensor("out", [VPC, T, D], f32, kind="ExternalOutput").ap()

    # which chunks of the weight-build run on gpsimd instead of DVE
    gp_chunks = set(range(0, NCH, gp_stride)) if gp_stride else set()

    with tile.TileContext(nc) as tc, ExitStack() as ctx:
        cpool = ctx.enter_context(tc.tile_pool(name="consts", bufs=1))
        tconst = cpool.tile([P, T + 1], f32, tag="tconst")
        ident = cpool.tile([P, P], f32, tag="ident")
        triex = cpool.tile([32, 32], f32, tag="triex")
        ones32 = cpool.tile([32, P], f32, tag="ones32")
        nc.sync.dma_start(tconst[:], tconst_d)
        nc.sync.dma_start(ident[:], ident_d)
        nc.sync.dma_start(triex[:], triex_d)
        nc.sync.dma_start(ones32[:], ones_d)

        fpool = ctx.enter_context(tc.tile_pool(name="feats", bufs=8))
        mpool = ctx.enter_context(tc.tile_pool(name="maskp", bufs=2))
        bpool = ctx.enter_context(tc.tile_pool(name="bound", bufs=2))
        wpool = ctx.enter_context(tc.tile_pool(name="wtile", bufs=4))
        opool = ctx.enter_context(tc.tile_pool(name="outsb", bufs=2))
        ps_ab = ctx.enter_context(tc.tile_pool(name="ps_ab", bufs=2, space="PSUM"))
        ps_tp = ctx.enter_context(tc.tile_pool(name="ps_tp", bufs=1, space="PSUM"))
        ps_out = ctx.enter_context(tc.tile_pool(name="ps_out", bufs=2, space="PSUM"))

        for v in range(VPC):
            # ---- feats DMA: [4096, 256] -> [128, 32, 256] (i = c*128 + p) ----
            fv = feats_d[v].rearrange("(c p) d -> p c d", p=P)  # [128, 32, 256]
            fq = []
            for q in range(4):
                fqt = fpool.tile([P, 8 * D], f32r, tag="F")
                nc.sync.dma_start(
                    fqt[:].rearrange("p (c d) -> p c d", d=D),
                    fv[:, q * 8 : (q + 1) * 8, :],
                )
                fq.append(fqt)

            # ---- mask pipeline ----
            bm8 = mpool.tile([32, P], u8, tag="bm8")
            nc.sync.dma_start(bm8[:], masks_d[v].rearrange("(c p) -> c p", p=P))
            bmf = mpool.tile([32, P], f32, tag="bmf")
            nc.vector.tensor_copy(bmf[:], bm8[:])  # u8 -> f32
            csum0 = mpool.tile([32, P], f32, tag="csum0")
            nc.vector.tensor_tensor_scan(
                csum0[:], ones32[:], bmf[:], 0.0, Alu.mult, Alu.add
            )
            tot = csum0[:, P - 1 : P]
            psab = ps_ab.tile([P, 2], f32, tag="ab")
            carry = psab[0:32, 0:1]
            nrep = psab[:, 1:2]
            nc.tensor.matmul(carry, triex[:], tot, start=True, stop=True)
            nc.tensor.matmul(nrep, ones32[:], tot, start=True, stop=True)
            cm = mpool.tile([32, P], f32, tag="cm")
            nc.vector.scalar_tensor_tensor(
                cm[:], csum0[:], carry, bmf[:], Alu.add, Alu.mult
            )
            cmT = ps_tp.tile([P, 128], f32, tag="tp")
            nc.tensor.transpose(cmT[:, 0:32], cm[:], ident[0:32, 0:32])
            ccol = mpool.tile([P, 32], f32, tag="ccol")
            nc.scalar.copy(ccol[:], cmT[:, 0:32])
            bias = mpool.tile([P, 32], f32, tag="bias")
            nc.vector.tensor_scalar(
                bias[:], ccol[:], -SIG, 0.5 * SIG, Alu.mult, Alu.add
            )
            nreps = mpool.tile([P, 1], f32, tag="nreps")
            nc.scalar.copy(nreps[:], nrep)
            gpe = nc.gpsimd if gp_misc else nc.vector
            nm1 = mpool.tile([P, 1], f32, tag="nm1")
            gpe.tensor_scalar(nm1[:], nreps[:], 1.0, None, Alu.subtract)

            # ---- boundaries: SALL[k] = min(rne(k*n/T), n-1), k = 0..T ----
            sr = bpool.tile([P, T + 1], f32, tag="sr")
            nc.scalar.activation(sr[:], tconst[:], Act.Copy, scale=nreps[:, 0:1])
            rnd = bpool.tile([P, T + 1], f32, tag="rnd")
            gpe.tensor_scalar(
                rnd[:], sr[:], TWO23, TWO23, Alu.add, Alu.subtract
            )
            sall = bpool.tile([P, T + 1], f32, tag="sall")
            gpe.tensor_scalar(sall[:], rnd[:], nm1[:, 0:1], None, Alu.min)
            hh = bpool.tile([P, T], f32, tag="hh")
            nc.vector.scalar_tensor_tensor(
                hh[:], sall[:, 0:T], 1.0, sall[:, 1 : T + 1], Alu.add, Alu.max
            )
            dd = bpool.tile([P, T], f32, tag="dd")
            gpe.tensor_tensor(dd[:], hh[:], sall[:, 0:T], Alu.subtract)
            invd = bpool.tile([P, 2], f32, tag="invd")
            for h in range(2):
                ddT = ps_tp.tile([P, 128], f32, tag="tp")
                nc.tensor.transpose(ddT[:], dd[:, h * P : (h + 1) * P], ident[:])
                nc.vector.reciprocal(invd[:, h : h + 1], ddT[:, 0:1])

            # ---- chunk loop: build W chunk, accumulate matmuls ----
            outps = [
                ps_out.tile([P, T], f32, tag="outps0", name=f"outps0_{v}"),
                ps_out.tile([P, T], f32, tag="outps1", name=f"outps1_{v}"),
            ]
            for c in range(NCH):
                g = wpool.tile([P, T], f32, tag="g")
                nc.scalar.activation(
                    g[:], hh[:], Act.Sigmoid, bias=bias[:, c : c + 1], scale=SIG
                )
                w = wpool.tile([P, T], f32r, tag="w")
                if c in gp_chunks:
                    lt = wpool.tile([P, T], f32, tag="lt")
                    nc.gpsimd.tensor_scalar(
                        lt[:], sall[:, 0:T], ccol[:, c : c + 1], None, Alu.is_lt
                    )
                    nc.gpsimd.tensor_tensor(w[:], lt[:], g[:], Alu.mult)
                else:
                    nc.vector.scalar_tensor_tensor(
                        w[:], sall[:, 0:T], ccol[:, c : c + 1], g[:],
                        Alu.is_lt, Alu.mult,
                    )
                fsrc = fq[c // 8][:, (c % 8) * D : (c % 8 + 1) * D]
                for h in range(2):
                    nc.tensor.matmul(
                        outps[h][:],
                        w[:, h * P : (h + 1) * P],
                        fsrc,
                        start=(c == 0),
                        stop=(c == NCH - 1),
                    )

            # ---- scale by 1/d and store ----
            outsb = opool.tile([P, 2 * T], f32, tag="outsb")
            for h in range(2):
                nc.scalar.activation(
                    outsb[:, h * T : (h + 1) * T],
                    outps[h][:],
                    Act.Copy,
                    scale=invd[:, h : h + 1],
                )
            ov = out_d[v].rearrange("(h p) d -> p h d", p=P)  # [128, 2, 256]
            nc.sync.dma_start(ov, outsb[:].rearrange("p (h d) -> p h d", d=D))

    nc.compile()
    return nc


def _consts():
    tconst = np.broadcast_to(
        (np.arange(T + 1, dtype=np.float32) / np.float32(T))[None, :], (P, T + 1)
    ).copy()
    ident = np.eye(P, dtype=np.float32)
    triex = np.triu(np.ones((32, 32), np.float32), 1)  # [q, p] = 1 if q < p
    ones32 = np.ones((32, P), np.float32)
    return tconst, ident, triex, ones32


def make_in_maps(feats: np.ndarray, masks: np.ndarray):
    tconst, ident, triex, ones32 = _consts()
    in_maps = []
    for c in range(N_CORES):
        in_maps.append(
            {
                "feats": np.ascontiguousarray(feats[c * VPC : (c + 1) * VPC]),
                "masks": np.ascontiguousarray(masks[c * VPC : (c + 1) * VPC]),
                "tconst": tconst,
                "ident": ident,
                "triex": triex,
                "ones32": ones32,
            }
        )
    return in_maps


def kernel(video_feats: np.ndarray, video_masks: np.ndarray) -> np.ndarray:
    from concourse.bass_utils import run_bass_kernel_spmd

    feats = np.ascontiguousarray(np.asarray(video_feats), dtype=np.float32)
    masks = np.ascontiguousarray(np.asarray(video_masks)).astype(np.uint8)
    assert feats.shape == (B, L, D) and masks.shape == (B, L)

    if "nc" not in _CACHE:
        _CACHE["nc"] = _build_program()
    nc = _CACHE["nc"]

    res = run_bass_kernel_spmd(nc, make_in_maps(feats, masks), list(range(N_CORES)))
    out = np.concatenate([r["out"] for r in res.results], axis=0)
    return out.astype(np.float32)
